# revision 8
# baseline (speedup 1.0000x reference)
"""Trainium2 Bass kernel for nn_AttentionHeadRankFour.

Computes, for inputs Xq, Xk, Xv [B=8, X=8, S=1024, D=512] and weights
Wq, Wk, Wv [512, 64]:
    Q = Xq Wq; K = Xk Wk; V = Xv Wv
    scores = Q K^T / sqrt(S), causal mask, softmax
    out = attn V                                  -> [8, 8, 1024, 64]

Sharding: data-parallel over batch B across 8 NeuronCores (core c gets b=c).
Weights replicated.

Per-core device program (everything in "transposed land" so the attention
matrix never needs an on-chip transpose):
  - load X tiles naturally [s,k], PE-transpose (exact fp32) -> X^T [k,s]
  - projections with W as the stationary operand -> Q^T, K^T, V^T [64, 1024]
    (padded to 128 partitions with zeros)
  - scores^T[sk,sq] = K @ Q^T block matmuls, fully-masked blocks skipped
  - attn^T = exp(scores^T/32) via ACT, causal edge blocks fixed by a gpsimd
    affine_select (fill 0 above the diagonal)
  - PV: lhsT = V_ext [sk-tile, 65] (V tile plus a ones column) so the
    softmax denominator accumulates as row 64 of the output psum
  - out^T [65, 1024] DMA'd out; host divides rows 0:64 by row 64 and
    transposes to [1024, 64]
Matmul dtype is switchable: exact float32 (4 cyc/row) or float32r
(1 cyc/row, reduced-precision multiply, fp32 accumulate). For float32r the
hardware requires producers to round on write, so matmul-feeding SBUF tiles
are stored as float32r (same 4-byte layout; the PSUM->SBUF copies do the
rounding for free).
"""

import numpy as np

P = 128
S = 1024
DK = 512
DO = 64
KO = DK // P          # 4 k-tiles of 128
ST = S // P           # 8 s-tiles of 128
CH = 512              # sq chunk width (psum bank)
NCH = S // CH         # 2
NCORES = 8
B = 8
X = 8

MM_DTYPE = "float32r"  # "float32" (exact, 4x slower PE) or "float32r"


def build_nc(nx: int = X, mm_dtype: str = MM_DTYPE):
    """Build the single-core Bass program processing nx x-slices."""
    from contextlib import ExitStack

    import concourse.bass as bass
    from concourse import bacc
    import concourse.tile as tile
    from concourse import mybir
    from concourse.masks import make_identity

    f32 = mybir.dt.float32
    mmdt = getattr(mybir.dt, mm_dtype)
    exact = mmdt == f32

    nc = bacc.Bacc("TRN2", target_bir_lowering=False, debug=False)

    xq = nc.dram_tensor("xq", [nx, S, DK], f32, kind="ExternalInput").ap()
    xk = nc.dram_tensor("xk", [nx, S, DK], f32, kind="ExternalInput").ap()
    xv = nc.dram_tensor("xv", [nx, S, DK], f32, kind="ExternalInput").ap()
    wq = nc.dram_tensor("wq", [DK, DO], f32, kind="ExternalInput").ap()
    wk = nc.dram_tensor("wk", [DK, DO], f32, kind="ExternalInput").ap()
    wv = nc.dram_tensor("wv", [DK, DO], f32, kind="ExternalInput").ap()
    out = nc.dram_tensor("out", [nx, DO + 1, S], f32, kind="ExternalOutput").ap()

    with tile.TileContext(nc) as tc, ExitStack() as ctx:
        const_pool = ctx.enter_context(tc.tile_pool(name="const", bufs=1))
        nat_pool = ctx.enter_context(tc.tile_pool(name="nat", bufs=12))
        xt_pool = ctx.enter_context(tc.tile_pool(name="xt", bufs=2))
        qkv_pool = ctx.enter_context(tc.tile_pool(name="qkv", bufs=2))
        vext_pool = ctx.enter_context(tc.tile_pool(name="vext", bufs=2))
        at_pool = ctx.enter_context(tc.tile_pool(name="at", bufs=4))
        outT_pool = ctx.enter_context(tc.tile_pool(name="outT", bufs=2))

        tp_ps = ctx.enter_context(tc.tile_pool(name="tp_ps", bufs=2, space="PSUM"))
        proj_ps = ctx.enter_context(tc.tile_pool(name="proj_ps", bufs=1, space="PSUM"))
        v_ps = ctx.enter_context(tc.tile_pool(name="v_ps", bufs=1, space="PSUM"))
        sc_ps = ctx.enter_context(tc.tile_pool(name="sc_ps", bufs=2, space="PSUM"))
        pv_ps = ctx.enter_context(tc.tile_pool(name="pv_ps", bufs=2, space="PSUM"))

        identity = const_pool.tile([P, P], f32, name="identity")
        make_identity(nc, identity[:])
        ones_col = const_pool.tile([P, 1], f32, name="ones_col")
        nc.gpsimd.memset(ones_col[:], 1.0)
        if exact:
            id_mm = identity[:]
        else:
            identity_r = const_pool.tile([P, P], mmdt, name="identity_r")
            nc.vector.tensor_copy(identity_r[:], identity[:])
            id_mm = identity_r[:]

        w_sb = {}
        for name, ap in (("wq", wq), ("wk", wk), ("wv", wv)):
            if exact:
                t = const_pool.tile([P, KO, DO], f32, name=f"{name}_sb")
                nc.sync.dma_start(t[:], ap.rearrange("(ko ki) d -> ki ko d", ki=P))
            else:
                raw = const_pool.tile([P, KO, DO], f32, name=f"{name}_raw")
                nc.sync.dma_start(raw[:], ap.rearrange("(ko ki) d -> ki ko d", ki=P))
                t = const_pool.tile([P, KO, DO], mmdt, name=f"{name}_sb")
                nc.vector.tensor_copy(t[:], raw[:])
            w_sb[name] = t

        for xi in range(nx):
            # --- load + transpose + project all three tensors ---
            pts = {}
            for tname, x_ap, wname in (("q", xq, "wq"), ("k", xk, "wk"), ("v", xv, "wv")):
                xt = xt_pool.tile([P, KO, S], mmdt, name="xt", tag="xt")
                for st in range(ST):
                    nat = nat_pool.tile([P, DK], f32, name="nat", tag="nat")
                    nc.sync.dma_start(nat[:], x_ap[xi, st * P:(st + 1) * P, :])
                    tp = tp_ps.tile([P, KO, P], f32, name="tp", tag="tp")
                    for ko in range(KO):
                        nc.tensor.transpose(
                            tp[:, ko, :], nat[:, ko * P:(ko + 1) * P], identity[:]
                        )
                    # one wide copy PSUM -> SBUF (rounds to mmdt), DVE/ACT split
                    if st % 2 == 0:
                        nc.vector.tensor_copy(xt[:, :, st * P:(st + 1) * P], tp[:])
                    else:
                        nc.scalar.copy(xt[:, :, st * P:(st + 1) * P], tp[:])

                # projection: P^T = W^T X^T  -> [64, S], padded to 128 parts
                pt = qkv_pool.tile([DO, S], mmdt, name=f"{tname}T", tag=f"{tname}T")
                for c in range(NCH):
                    pp = proj_ps.tile([DO, CH], f32, name="pp", tag="pp")
                    for ko in range(KO):
                        nc.tensor.matmul(
                            pp[:],
                            w_sb[wname][:, ko, :],
                            xt[:, ko, c * CH:(c + 1) * CH],
                            start=(ko == 0),
                            stop=(ko == KO - 1),
                        )
                    if c % 2 == 0:
                        nc.vector.tensor_copy(pt[:DO, c * CH:(c + 1) * CH], pp[:])
                    else:
                        nc.scalar.copy(pt[:DO, c * CH:(c + 1) * CH], pp[:])
                pts[tname] = pt

            # --- V_ext tiles: [128 sk, 65] = [V tile | ones] ---
            vext = vext_pool.tile([P, ST, DO + 1], mmdt, name="vext", tag="vext")
            nc.vector.tensor_copy(vext[:, :, DO], ones_col[:].to_broadcast([P, ST]))
            for i in range(ST):
                vp = v_ps.tile([P, DO], mmdt, name="vp", tag="vp")
                nc.tensor.transpose(
                    vp[:], pts["v"][:, i * P:(i + 1) * P], id_mm[:DO, :DO]
                )
                nc.vector.tensor_copy(vext[:, i, :DO], vp[:])

            # --- attention ---
            outT = outT_pool.tile([DO + 1, S], f32, name="outT", tag="outT")
            for j in range(NCH):
                ilast = min(ST - 1, 4 * j + 3)
                pv = pv_ps.tile([DO + 1, CH], f32, name="pv", tag="pv")
                for i in range(ilast + 1):
                    sc = sc_ps.tile([P, CH], f32, name="sc", tag="sc")
                    nc.tensor.matmul(
                        sc[:],
                        pts["k"][:, i * P:(i + 1) * P],
                        pts["q"][:, j * CH:(j + 1) * CH],
                        start=True,
                        stop=True,
                    )
                    at = at_pool.tile([P, CH], mmdt, name="at", tag="at")
                    nc.scalar.activation(
                        at[:], sc[:],
                        mybir.ActivationFunctionType.Exp,
                        scale=1.0 / 32.0,
                    )
                    if i >= 4 * j:  # block straddles the diagonal
                        # keep at[r, c] where (512j + c) - (128i + r) >= 0
                        nc.gpsimd.affine_select(
                            at[:], at[:],
                            pattern=[[1, CH]],
                            compare_op=mybir.AluOpType.is_ge,
                            fill=0.0,
                            base=CH * j - P * i,
                            channel_multiplier=-1,
                        )
                    nc.tensor.matmul(
                        pv[:],
                        vext[:, i, :],
                        at[:],
                        start=(i == 0),
                        stop=(i == ilast),
                    )
                if j % 2 == 0:
                    nc.vector.tensor_copy(outT[:, j * CH:(j + 1) * CH], pv[:])
                else:
                    nc.scalar.copy(outT[:, j * CH:(j + 1) * CH], pv[:])
            nc.sync.dma_start(out[xi], outT[:])

    nc.compile()
    return nc


_NC_CACHE: dict = {}


def _get_nc(nx: int = X, mm_dtype: str = MM_DTYPE):
    key = (nx, mm_dtype)
    if key not in _NC_CACHE:
        _NC_CACHE[key] = build_nc(nx, mm_dtype)
    return _NC_CACHE[key]


def run_device(inputs: dict, mm_dtype: str = MM_DTYPE, trace: bool = False, **kw):
    """Run the SPMD kernel on 8 cores; returns (per-core raw outs, BassKernelResults)."""
    from concourse import bass_utils

    xq = np.ascontiguousarray(np.asarray(inputs["inputs_for_queries"], np.float32))
    xk = np.ascontiguousarray(np.asarray(inputs["inputs_for_keys"], np.float32))
    xv = np.ascontiguousarray(np.asarray(inputs["inputs_for_values"], np.float32))
    wq = np.ascontiguousarray(np.asarray(inputs["q_weight"], np.float32))
    wk = np.ascontiguousarray(np.asarray(inputs["k_weight"], np.float32))
    wv = np.ascontiguousarray(np.asarray(inputs["v_weight"], np.float32))

    nc = _get_nc(X, mm_dtype)
    in_maps = [
        {"xq": xq[b], "xk": xk[b], "xv": xv[b], "wq": wq, "wk": wk, "wv": wv}
        for b in range(NCORES)
    ]
    res = bass_utils.run_bass_kernel_spmd(
        nc, in_maps, core_ids=list(range(NCORES)), trace=trace, **kw
    )
    outs = [res.results[b]["out"] for b in range(NCORES)]
    return outs, res


def finalize(outs) -> np.ndarray:
    """Host-side: normalize by the softmax denominator and un-transpose."""
    full = np.stack(outs, axis=0)                      # [B, X, 65, S]
    o = full[:, :, :DO, :] / full[:, :, DO:DO + 1, :]  # [B, X, 64, S]
    return np.ascontiguousarray(o.transpose(0, 1, 3, 2)).astype(np.float32)


def kernel(**inputs) -> np.ndarray:
    outs, _ = run_device(inputs)
    return finalize(outs)


if __name__ == "__main__":
    nc = build_nc(1)
    print("built ok")


# revision 9
# speedup vs baseline: 1.0898x; 1.0898x over previous
"""Trainium2 Bass kernel for nn_AttentionHeadRankFour.

Computes, for inputs Xq, Xk, Xv [B=8, X=8, S=1024, D=512] and weights
Wq, Wk, Wv [512, 64]:
    Q = Xq Wq; K = Xk Wk; V = Xv Wv
    scores = Q K^T / sqrt(S), causal mask, softmax
    out = attn V                                  -> [8, 8, 1024, 64]

Sharding: data-parallel over batch B across 8 NeuronCores (core c gets b=c).
Weights replicated.

Per-core device program, in "transposed land" so the attention matrix never
needs an on-chip transpose:
  - X^T [k, s] tiles on chip (per-mode, see below)
  - projections with W as the stationary operand -> Q^T, K^T, V^T [64, 1024]
  - scores^T[sk,sq] = K @ Q^T block matmuls (float32r), fully-masked blocks
    skipped
  - attn^T = exp(scores^T/32) via ACT (the 1/sqrt(S) folds into the
    activation scale), causal edge blocks fixed by a gpsimd affine_select
  - PV: lhsT = V_ext [sk-tile, 65] (V tile plus a ones column) so the
    softmax denominator accumulates as row 64 of the output psum
  - out^T [65, 1024] DMA'd out; host divides rows 0:64 by row 64 and
    transposes to [1024, 64]

Modes (how X^T is produced / projection precision):
  - "f32r": inputs fp32, natural loads + PE transposes (exact), projections
    in float32r (fp32 storage, ~2^-14 multiply rounding, fp32 accumulate).
  - "bf16": inputs pre-cast to bf16 on host in [KO, S, 128] k-chunked
    layout; X^T produced directly by XBAR DMA-transpose loads (no PE
    transposes, no PSUM->SBUF copies); projections in bf16; attention
    stays float32r.
"""

import numpy as np

P = 128
S = 1024
DK = 512
DO = 64
KO = DK // P          # 4 k-tiles of 128
ST = S // P           # 8 s-tiles of 128
CH = 512              # sq chunk width (psum bank)
NCH = S // CH         # 2
NCORES = 8
B = 8
X = 8

MODE = "bf16"          # "f32r" or "bf16"


def build_nc(nx: int = X, mode: str = MODE):
    """Build the single-core Bass program processing nx x-slices."""
    from contextlib import ExitStack

    import concourse.tile as tile
    from concourse import bacc, mybir
    from concourse.masks import make_identity

    f32 = mybir.dt.float32
    f32r = mybir.dt.float32r
    bf16 = mybir.dt.bfloat16

    nc = bacc.Bacc("TRN2", target_bir_lowering=False, debug=False)

    if mode == "bf16":
        xq = nc.dram_tensor("xq", [nx, KO, S, P], bf16, kind="ExternalInput").ap()
        xk = nc.dram_tensor("xk", [nx, KO, S, P], bf16, kind="ExternalInput").ap()
        xv = nc.dram_tensor("xv", [nx, KO, S, P], bf16, kind="ExternalInput").ap()
        xdt = bf16
    else:
        xq = nc.dram_tensor("xq", [nx, S, DK], f32, kind="ExternalInput").ap()
        xk = nc.dram_tensor("xk", [nx, S, DK], f32, kind="ExternalInput").ap()
        xv = nc.dram_tensor("xv", [nx, S, DK], f32, kind="ExternalInput").ap()
        xdt = f32r
    wq = nc.dram_tensor("wq", [DK, DO], f32, kind="ExternalInput").ap()
    wk = nc.dram_tensor("wk", [DK, DO], f32, kind="ExternalInput").ap()
    wv = nc.dram_tensor("wv", [DK, DO], f32, kind="ExternalInput").ap()
    out = nc.dram_tensor("out", [nx, DO + 1, S], f32, kind="ExternalOutput").ap()

    with tile.TileContext(nc) as tc, ExitStack() as ctx:
        const_pool = ctx.enter_context(tc.tile_pool(name="const", bufs=1))
        xt_pool = ctx.enter_context(tc.tile_pool(name="xt", bufs=3))
        qkv_pool = ctx.enter_context(tc.tile_pool(name="qkv", bufs=2))
        vext_pool = ctx.enter_context(tc.tile_pool(name="vext", bufs=2))
        at_pool = ctx.enter_context(tc.tile_pool(name="at", bufs=4))
        outT_pool = ctx.enter_context(tc.tile_pool(name="outT", bufs=2))

        proj_ps = ctx.enter_context(tc.tile_pool(name="proj_ps", bufs=2, space="PSUM"))
        v_ps = ctx.enter_context(tc.tile_pool(name="v_ps", bufs=2, space="PSUM"))
        sc_ps = ctx.enter_context(tc.tile_pool(name="sc_ps", bufs=2, space="PSUM"))
        pv_ps = ctx.enter_context(tc.tile_pool(name="pv_ps", bufs=2, space="PSUM"))
        if mode != "bf16":
            nat_pool = ctx.enter_context(tc.tile_pool(name="nat", bufs=12))
            tp_ps = ctx.enter_context(tc.tile_pool(name="tp_ps", bufs=2, space="PSUM"))

        identity = const_pool.tile([P, P], f32, name="identity")
        make_identity(nc, identity[:])
        ones_col = const_pool.tile([P, 1], f32, name="ones_col")
        nc.gpsimd.memset(ones_col[:], 1.0)
        identity_r = const_pool.tile([P, P], f32r, name="identity_r")
        nc.vector.tensor_copy(identity_r[:], identity[:])
        id_mm = identity_r[:]

        w_sb = {}
        for name, ap in (("wq", wq), ("wk", wk), ("wv", wv)):
            raw = const_pool.tile([P, KO, DO], f32, name=f"{name}_raw")
            nc.gpsimd.dma_start(raw[:], ap.rearrange("(ko ki) d -> ki ko d", ki=P))
            t = const_pool.tile([P, KO, DO], xdt, name=f"{name}_sb")
            nc.vector.tensor_copy(t[:], raw[:])
            w_sb[name] = t

        for xi in range(nx):
            # --- X^T tiles + projections for all three tensors ---
            pts = {}
            for tname, x_ap, wname in (("q", xq, "wq"), ("k", xk, "wk"), ("v", xv, "wv")):
                xt = xt_pool.tile([P, KO, S], xdt, name="xt", tag="xt")
                if mode == "bf16":
                    for ko in range(KO):
                        nc.sync.dma_start_transpose(xt[:, ko, :], x_ap[xi, ko])
                else:
                    for st in range(ST):
                        nat = nat_pool.tile([P, DK], f32, name="nat", tag="nat")
                        nc.sync.dma_start(nat[:], x_ap[xi, st * P:(st + 1) * P, :])
                        tp = tp_ps.tile([P, KO, P], f32, name="tp", tag="tp")
                        for ko in range(KO):
                            nc.tensor.transpose(
                                tp[:, ko, :], nat[:, ko * P:(ko + 1) * P], identity[:]
                            )
                        # one wide copy PSUM -> SBUF (rounds to f32r)
                        if st % 2 == 0:
                            nc.vector.tensor_copy(xt[:, :, st * P:(st + 1) * P], tp[:])
                        else:
                            nc.scalar.copy(xt[:, :, st * P:(st + 1) * P], tp[:])

                # projection: P^T = W^T X^T  -> [64, S] float32r
                pt = qkv_pool.tile([DO, S], f32r, name=f"{tname}T", tag=f"{tname}T")
                for c in range(NCH):
                    pp = proj_ps.tile([DO, CH], f32, name="pp", tag="pp")
                    for ko in range(KO):
                        nc.tensor.matmul(
                            pp[:],
                            w_sb[wname][:, ko, :],
                            xt[:, ko, c * CH:(c + 1) * CH],
                            start=(ko == 0),
                            stop=(ko == KO - 1),
                        )
                    if c % 2 == 0:
                        nc.vector.tensor_copy(pt[:, c * CH:(c + 1) * CH], pp[:])
                    else:
                        nc.scalar.copy(pt[:, c * CH:(c + 1) * CH], pp[:])
                pts[tname] = pt

            # --- V_ext tiles: [128 sk, 65] = [V tile | ones] ---
            vext = vext_pool.tile([P, ST, DO + 1], f32r, name="vext", tag="vext")
            nc.vector.tensor_copy(vext[:, :, DO], ones_col[:].to_broadcast([P, ST]))
            for i in range(ST):
                vp = v_ps.tile([P, DO], f32r, name="vp", tag="vp")
                nc.tensor.transpose(
                    vp[:], pts["v"][:, i * P:(i + 1) * P], id_mm[:DO, :DO]
                )
                nc.vector.tensor_copy(vext[:, i, :DO], vp[:])

            # --- attention ---
            outT = outT_pool.tile([DO + 1, S], f32, name="outT", tag="outT")
            for j in range(NCH):
                ilast = min(ST - 1, 4 * j + 3)
                pv = pv_ps.tile([DO + 1, CH], f32, name="pv", tag="pv")
                for i in range(ilast + 1):
                    sc = sc_ps.tile([P, CH], f32, name="sc", tag="sc")
                    nc.tensor.matmul(
                        sc[:],
                        pts["k"][:, i * P:(i + 1) * P],
                        pts["q"][:, j * CH:(j + 1) * CH],
                        start=True,
                        stop=True,
                    )
                    at = at_pool.tile([P, CH], f32r, name="at", tag="at")
                    nc.scalar.activation(
                        at[:], sc[:],
                        mybir.ActivationFunctionType.Exp,
                        scale=1.0 / 32.0,
                    )
                    if i >= 4 * j:  # block straddles the diagonal
                        # keep at[r, c] where (512j + c) - (128i + r) >= 0
                        nc.gpsimd.affine_select(
                            at[:], at[:],
                            pattern=[[1, CH]],
                            compare_op=mybir.AluOpType.is_ge,
                            fill=0.0,
                            base=CH * j - P * i,
                            channel_multiplier=-1,
                        )
                    nc.tensor.matmul(
                        pv[:],
                        vext[:, i, :],
                        at[:],
                        start=(i == 0),
                        stop=(i == ilast),
                    )
                if j % 2 == 0:
                    nc.vector.tensor_copy(outT[:, j * CH:(j + 1) * CH], pv[:])
                else:
                    nc.scalar.copy(outT[:, j * CH:(j + 1) * CH], pv[:])
            nc.gpsimd.dma_start(out[xi], outT[:])

    nc.compile()
    return nc


_NC_CACHE: dict = {}


def _get_nc(nx: int = X, mode: str = MODE):
    key = (nx, mode)
    if key not in _NC_CACHE:
        _NC_CACHE[key] = build_nc(nx, mode)
    return _NC_CACHE[key]


def prep_x(x: np.ndarray, mode: str):
    """Host-side input prep for one [..., S, DK] tensor."""
    import ml_dtypes

    if mode == "bf16":
        # [..., S, DK] -> bf16 [..., KO, S, 128] (contiguous k-chunks)
        xb = x.astype(ml_dtypes.bfloat16)
        xb = xb.reshape(x.shape[:-1] + (KO, P))
        xb = np.moveaxis(xb, -2, -3)
        return np.ascontiguousarray(xb)
    return np.ascontiguousarray(x.astype(np.float32))


def run_device(inputs: dict, mode: str = MODE, trace: bool = False, **kw):
    """Run the SPMD kernel on 8 cores; returns (per-core raw outs, BassKernelResults)."""
    from concourse import bass_utils

    xq = prep_x(np.asarray(inputs["inputs_for_queries"], np.float32), mode)
    xk = prep_x(np.asarray(inputs["inputs_for_keys"], np.float32), mode)
    xv = prep_x(np.asarray(inputs["inputs_for_values"], np.float32), mode)
    wq = np.ascontiguousarray(np.asarray(inputs["q_weight"], np.float32))
    wk = np.ascontiguousarray(np.asarray(inputs["k_weight"], np.float32))
    wv = np.ascontiguousarray(np.asarray(inputs["v_weight"], np.float32))

    nc = _get_nc(X, mode)
    in_maps = [
        {"xq": xq[b], "xk": xk[b], "xv": xv[b], "wq": wq, "wk": wk, "wv": wv}
        for b in range(NCORES)
    ]
    res = bass_utils.run_bass_kernel_spmd(
        nc, in_maps, core_ids=list(range(NCORES)), trace=trace, **kw
    )
    outs = [res.results[b]["out"] for b in range(NCORES)]
    return outs, res


def finalize(outs) -> np.ndarray:
    """Host-side: normalize by the softmax denominator and un-transpose."""
    full = np.stack(outs, axis=0)                      # [B, X, 65, S]
    o = full[:, :, :DO, :] / full[:, :, DO:DO + 1, :]  # [B, X, 64, S]
    return np.ascontiguousarray(o.transpose(0, 1, 3, 2)).astype(np.float32)


def kernel(**inputs) -> np.ndarray:
    outs, _ = run_device(inputs)
    return finalize(outs)


if __name__ == "__main__":
    nc = build_nc(1)
    print("built ok")


# revision 10
# speedup vs baseline: 1.1302x; 1.0371x over previous
"""Trainium2 Bass kernel for nn_AttentionHeadRankFour.

Computes, for inputs Xq, Xk, Xv [B=8, X=8, S=1024, D=512] and weights
Wq, Wk, Wv [512, 64]:
    Q = Xq Wq; K = Xk Wk; V = Xv Wv
    scores = Q K^T / sqrt(S), causal mask, softmax
    out = attn V                                  -> [8, 8, 1024, 64]

Sharding: data-parallel over batch B across 8 NeuronCores (core c gets b=c).
Weights replicated.

Per-core device program, in "transposed land" so the attention matrix never
needs an on-chip transpose:
  - X^T [k, s] tiles on chip (per-mode, see below)
  - projections with W as the stationary operand -> Q^T, K^T, V^T [64, 1024]
  - scores^T[sk,sq] = K @ Q^T block matmuls (float32r), fully-masked blocks
    skipped
  - attn^T = exp(scores^T/32) via ACT (the 1/sqrt(S) folds into the
    activation scale), causal edge blocks fixed by a gpsimd affine_select
  - PV: lhsT = V_ext [sk-tile, 65] (V tile plus a ones column) so the
    softmax denominator accumulates as row 64 of the output psum
  - out^T [65, 1024] DMA'd out; host divides rows 0:64 by row 64 and
    transposes to [1024, 64]

Modes (how X^T is produced / projection precision):
  - "f32r": inputs fp32, natural loads + PE transposes (exact), projections
    in float32r (fp32 storage, ~2^-14 multiply rounding, fp32 accumulate).
  - "bf16": inputs pre-cast to bf16 on host in [KO, S, 128] k-chunked
    layout; X^T produced directly by XBAR DMA-transpose loads (no PE
    transposes, no PSUM->SBUF copies); projections in bf16; attention
    stays float32r.
"""

import numpy as np

P = 128
S = 1024
DK = 512
DO = 64
KO = DK // P          # 4 k-tiles of 128
ST = S // P           # 8 s-tiles of 128
CH = 512              # sq chunk width (psum bank)
NCH = S // CH         # 2
NCORES = 8
B = 8
X = 8

MODE = "bf16"          # "f32r" or "bf16"


def build_nc(nx: int = X, mode: str = MODE):
    """Build the single-core Bass program processing nx x-slices."""
    from contextlib import ExitStack

    import concourse.tile as tile
    from concourse import bacc, mybir
    from concourse.masks import make_identity

    f32 = mybir.dt.float32
    f32r = mybir.dt.float32r
    bf16 = mybir.dt.bfloat16

    nc = bacc.Bacc("TRN2", target_bir_lowering=False, debug=False)

    if mode == "bf16":
        xq = nc.dram_tensor("xq", [nx, S, DK], bf16, kind="ExternalInput").ap()
        xk = nc.dram_tensor("xk", [nx, S, DK], bf16, kind="ExternalInput").ap()
        xv = nc.dram_tensor("xv", [nx, S, DK], bf16, kind="ExternalInput").ap()
        xdt = bf16
    else:
        xq = nc.dram_tensor("xq", [nx, S, DK], f32, kind="ExternalInput").ap()
        xk = nc.dram_tensor("xk", [nx, S, DK], f32, kind="ExternalInput").ap()
        xv = nc.dram_tensor("xv", [nx, S, DK], f32, kind="ExternalInput").ap()
        xdt = f32r
    wq = nc.dram_tensor("wq", [DK, DO], f32, kind="ExternalInput").ap()
    wk = nc.dram_tensor("wk", [DK, DO], f32, kind="ExternalInput").ap()
    wv = nc.dram_tensor("wv", [DK, DO], f32, kind="ExternalInput").ap()
    out = nc.dram_tensor("out", [nx, DO + 1, S], f32, kind="ExternalOutput").ap()

    with tile.TileContext(nc) as tc, ExitStack() as ctx:
        const_pool = ctx.enter_context(tc.tile_pool(name="const", bufs=1))
        xt_pool = ctx.enter_context(tc.tile_pool(name="xt", bufs=3))
        qkv_pool = ctx.enter_context(tc.tile_pool(name="qkv", bufs=2))
        vext_pool = ctx.enter_context(tc.tile_pool(name="vext", bufs=2))
        at_pool = ctx.enter_context(tc.tile_pool(name="at", bufs=4))
        outT_pool = ctx.enter_context(tc.tile_pool(name="outT", bufs=2))

        proj_ps = ctx.enter_context(tc.tile_pool(name="proj_ps", bufs=2, space="PSUM"))
        v_ps = ctx.enter_context(tc.tile_pool(name="v_ps", bufs=2, space="PSUM"))
        sc_ps = ctx.enter_context(tc.tile_pool(name="sc_ps", bufs=2, space="PSUM"))
        pv_ps = ctx.enter_context(tc.tile_pool(name="pv_ps", bufs=2, space="PSUM"))
        if mode != "bf16":
            nat_pool = ctx.enter_context(tc.tile_pool(name="nat", bufs=12))
            tp_ps = ctx.enter_context(tc.tile_pool(name="tp_ps", bufs=2, space="PSUM"))

        identity = const_pool.tile([P, P], f32, name="identity")
        make_identity(nc, identity[:])
        ones_col = const_pool.tile([P, 1], f32, name="ones_col")
        nc.gpsimd.memset(ones_col[:], 1.0)
        identity_r = const_pool.tile([P, P], f32r, name="identity_r")
        nc.vector.tensor_copy(identity_r[:], identity[:])
        id_mm = identity_r[:]

        # causal 0/1 masks for the 4 diagonal-straddling block offsets:
        # masks[m][r, c] = 1.0 iff c - r - 128*m >= 0
        masks = []
        for m in range(4):
            mk_raw = const_pool.tile([P, CH], f32, name=f"mask{m}_raw")
            nc.gpsimd.memset(mk_raw[:], 1.0)
            nc.gpsimd.affine_select(
                mk_raw[:], mk_raw[:],
                pattern=[[1, CH]],
                compare_op=mybir.AluOpType.is_ge,
                fill=0.0,
                base=-P * m,
                channel_multiplier=-1,
            )
            mk = const_pool.tile([P, CH], f32r, name=f"mask{m}")
            nc.vector.tensor_copy(mk[:], mk_raw[:])
            masks.append(mk)

        w_sb = {}
        for name, ap in (("wq", wq), ("wk", wk), ("wv", wv)):
            raw = const_pool.tile([P, KO, DO], f32, name=f"{name}_raw")
            nc.gpsimd.dma_start(raw[:], ap.rearrange("(ko ki) d -> ki ko d", ki=P))
            t = const_pool.tile([P, KO, DO], xdt, name=f"{name}_sb")
            nc.vector.tensor_copy(t[:], raw[:])
            w_sb[name] = t

        for xi in range(nx):
            # --- X^T tiles + projections for all three tensors ---
            pts = {}
            for tname, x_ap, wname in (("q", xq, "wq"), ("k", xk, "wk"), ("v", xv, "wv")):
                xt = xt_pool.tile([P, KO, S], xdt, name="xt", tag="xt")
                if mode == "bf16":
                    nc.sync.dma_start_transpose(xt[:], x_ap[xi])
                else:
                    for st in range(ST):
                        nat = nat_pool.tile([P, DK], f32, name="nat", tag="nat")
                        nc.sync.dma_start(nat[:], x_ap[xi, st * P:(st + 1) * P, :])
                        tp = tp_ps.tile([P, KO, P], f32, name="tp", tag="tp")
                        for ko in range(KO):
                            nc.tensor.transpose(
                                tp[:, ko, :], nat[:, ko * P:(ko + 1) * P], identity[:]
                            )
                        # one wide copy PSUM -> SBUF (rounds to f32r)
                        if st % 2 == 0:
                            nc.vector.tensor_copy(xt[:, :, st * P:(st + 1) * P], tp[:])
                        else:
                            nc.scalar.copy(xt[:, :, st * P:(st + 1) * P], tp[:])

                # projection: P^T = W^T X^T  -> [64, S] float32r
                pt = qkv_pool.tile([DO, S], f32r, name=f"{tname}T", tag=f"{tname}T")
                for c in range(NCH):
                    pp = proj_ps.tile([DO, CH], f32, name="pp", tag="pp")
                    for ko in range(KO):
                        nc.tensor.matmul(
                            pp[:],
                            w_sb[wname][:, ko, :],
                            xt[:, ko, c * CH:(c + 1) * CH],
                            start=(ko == 0),
                            stop=(ko == KO - 1),
                        )
                    if c % 2 == 0:
                        nc.vector.tensor_copy(pt[:, c * CH:(c + 1) * CH], pp[:])
                    else:
                        nc.scalar.copy(pt[:, c * CH:(c + 1) * CH], pp[:])
                pts[tname] = pt

            # --- V_ext tiles: [128 sk, 65] = [V tile | ones] ---
            vext = vext_pool.tile([P, ST, DO + 1], f32r, name="vext", tag="vext")
            nc.vector.tensor_copy(vext[:, :, DO], ones_col[:].to_broadcast([P, ST]))
            for i in range(ST):
                vp = v_ps.tile([P, DO], f32r, name="vp", tag="vp")
                nc.tensor.transpose(
                    vp[:], pts["v"][:, i * P:(i + 1) * P], id_mm[:DO, :DO]
                )
                nc.vector.tensor_copy(vext[:, i, :DO], vp[:])

            # --- attention ---
            outT = outT_pool.tile([DO + 1, S], f32, name="outT", tag="outT")
            for j in range(NCH):
                ilast = min(ST - 1, 4 * j + 3)
                pv = pv_ps.tile([DO + 1, CH], f32, name="pv", tag="pv")
                for i in range(ilast + 1):
                    sc = sc_ps.tile([P, CH], f32, name="sc", tag="sc")
                    nc.tensor.matmul(
                        sc[:],
                        pts["k"][:, i * P:(i + 1) * P],
                        pts["q"][:, j * CH:(j + 1) * CH],
                        start=True,
                        stop=True,
                    )
                    at = at_pool.tile([P, CH], f32r, name="at", tag="at")
                    nc.scalar.activation(
                        at[:], sc[:],
                        mybir.ActivationFunctionType.Exp,
                        scale=1.0 / 32.0,
                    )
                    if i >= 4 * j:  # block straddles the diagonal
                        nc.vector.tensor_tensor(
                            at[:], at[:], masks[i - 4 * j][:],
                            mybir.AluOpType.mult,
                        )
                    nc.tensor.matmul(
                        pv[:],
                        vext[:, i, :],
                        at[:],
                        start=(i == 0),
                        stop=(i == ilast),
                    )
                if j % 2 == 0:
                    nc.vector.tensor_copy(outT[:, j * CH:(j + 1) * CH], pv[:])
                else:
                    nc.scalar.copy(outT[:, j * CH:(j + 1) * CH], pv[:])
            nc.gpsimd.dma_start(out[xi], outT[:])

    nc.compile()
    return nc


_NC_CACHE: dict = {}


def _get_nc(nx: int = X, mode: str = MODE):
    key = (nx, mode)
    if key not in _NC_CACHE:
        _NC_CACHE[key] = build_nc(nx, mode)
    return _NC_CACHE[key]


def prep_x(x: np.ndarray, mode: str):
    """Host-side input prep for one [..., S, DK] tensor."""
    import ml_dtypes

    if mode == "bf16":
        return np.ascontiguousarray(x.astype(ml_dtypes.bfloat16))
    return np.ascontiguousarray(x.astype(np.float32))


def run_device(inputs: dict, mode: str = MODE, trace: bool = False, **kw):
    """Run the SPMD kernel on 8 cores; returns (per-core raw outs, BassKernelResults)."""
    from concourse import bass_utils

    xq = prep_x(np.asarray(inputs["inputs_for_queries"], np.float32), mode)
    xk = prep_x(np.asarray(inputs["inputs_for_keys"], np.float32), mode)
    xv = prep_x(np.asarray(inputs["inputs_for_values"], np.float32), mode)
    wq = np.ascontiguousarray(np.asarray(inputs["q_weight"], np.float32))
    wk = np.ascontiguousarray(np.asarray(inputs["k_weight"], np.float32))
    wv = np.ascontiguousarray(np.asarray(inputs["v_weight"], np.float32))

    nc = _get_nc(X, mode)
    in_maps = [
        {"xq": xq[b], "xk": xk[b], "xv": xv[b], "wq": wq, "wk": wk, "wv": wv}
        for b in range(NCORES)
    ]
    res = bass_utils.run_bass_kernel_spmd(
        nc, in_maps, core_ids=list(range(NCORES)), trace=trace, **kw
    )
    outs = [res.results[b]["out"] for b in range(NCORES)]
    return outs, res


def finalize(outs) -> np.ndarray:
    """Host-side: normalize by the softmax denominator and un-transpose."""
    full = np.stack(outs, axis=0)                      # [B, X, 65, S]
    o = full[:, :, :DO, :] / full[:, :, DO:DO + 1, :]  # [B, X, 64, S]
    return np.ascontiguousarray(o.transpose(0, 1, 3, 2)).astype(np.float32)


def kernel(**inputs) -> np.ndarray:
    outs, _ = run_device(inputs)
    return finalize(outs)


if __name__ == "__main__":
    nc = build_nc(1)
    print("built ok")


# revision 12
# speedup vs baseline: 1.2564x; 1.1116x over previous
"""Trainium2 Bass kernel for nn_AttentionHeadRankFour.

Computes, for inputs Xq, Xk, Xv [B=8, X=8, S=1024, D=512] and weights
Wq, Wk, Wv [512, 64]:
    Q = Xq Wq; K = Xk Wk; V = Xv Wv
    scores = Q K^T / sqrt(S), causal mask, softmax
    out = attn V                                  -> [8, 8, 1024, 64]

Sharding: data-parallel over batch B across 8 NeuronCores (core c gets b=c).
Weights replicated.

Per-core device program, in "transposed land" so the attention matrix never
needs an on-chip transpose:
  - X^T [k, s] tiles on chip (per-mode, see below)
  - projections with W as the stationary operand -> Q^T, K^T, V^T [64, 1024]
  - scores^T[sk,sq] = K @ Q^T block matmuls (float32r), fully-masked blocks
    skipped
  - attn^T = exp(scores^T/32) via ACT (the 1/sqrt(S) folds into the
    activation scale), causal edge blocks fixed by a gpsimd affine_select
  - PV: lhsT = V_ext [sk-tile, 65] (V tile plus a ones column) so the
    softmax denominator accumulates as row 64 of the output psum
  - out^T [65, 1024] DMA'd out; host divides rows 0:64 by row 64 and
    transposes to [1024, 64]

Modes (how X^T is produced / projection precision):
  - "f32r": inputs fp32, natural loads + PE transposes (exact), projections
    in float32r (fp32 storage, ~2^-14 multiply rounding, fp32 accumulate).
  - "bf16": inputs pre-cast to bf16 on host in [KO, S, 128] k-chunked
    layout; X^T produced directly by XBAR DMA-transpose loads (no PE
    transposes, no PSUM->SBUF copies); projections in bf16; attention
    stays float32r.
"""

import numpy as np

P = 128
S = 1024
DK = 512
DO = 64
KO = DK // P          # 4 k-tiles of 128
ST = S // P           # 8 s-tiles of 128
CH = 512              # sq chunk width (psum bank)
NCH = S // CH         # 2
NCORES = 8
B = 8
X = 8

MODE = "bf16"          # "f32r" or "bf16"


def build_nc(nx: int = X, mode: str = MODE):
    """Build the single-core Bass program processing nx x-slices."""
    from contextlib import ExitStack

    import concourse.tile as tile
    from concourse import bacc, mybir
    from concourse.masks import make_identity

    f32 = mybir.dt.float32
    f32r = mybir.dt.float32r
    bf16 = mybir.dt.bfloat16

    nc = bacc.Bacc("TRN2", target_bir_lowering=False, debug=False)

    if mode == "bf16":
        xq = nc.dram_tensor("xq", [nx, S, DK], bf16, kind="ExternalInput").ap()
        xk = nc.dram_tensor("xk", [nx, S, DK], bf16, kind="ExternalInput").ap()
        xv = nc.dram_tensor("xv", [nx, S, DK], bf16, kind="ExternalInput").ap()
        xdt = bf16
    else:
        xq = nc.dram_tensor("xq", [nx, S, DK], f32, kind="ExternalInput").ap()
        xk = nc.dram_tensor("xk", [nx, S, DK], f32, kind="ExternalInput").ap()
        xv = nc.dram_tensor("xv", [nx, S, DK], f32, kind="ExternalInput").ap()
        xdt = f32r
    wq = nc.dram_tensor("wq", [DK, DO], f32, kind="ExternalInput").ap()
    wk = nc.dram_tensor("wk", [DK, DO], f32, kind="ExternalInput").ap()
    wv = nc.dram_tensor("wv", [DK, DO], f32, kind="ExternalInput").ap()
    out = nc.dram_tensor("out", [nx, DO + 1, S], f32, kind="ExternalOutput").ap()

    with tile.TileContext(nc) as tc, ExitStack() as ctx:
        const_pool = ctx.enter_context(tc.tile_pool(name="const", bufs=1))
        xt_pool = ctx.enter_context(tc.tile_pool(name="xt", bufs=6))
        qkv_pool = ctx.enter_context(tc.tile_pool(name="qkv", bufs=2))
        vext_pool = ctx.enter_context(tc.tile_pool(name="vext", bufs=2))
        at_pool = ctx.enter_context(tc.tile_pool(name="at", bufs=4))
        outT_pool = ctx.enter_context(tc.tile_pool(name="outT", bufs=2))

        proj_ps = ctx.enter_context(tc.tile_pool(name="proj_ps", bufs=2, space="PSUM"))
        v_ps = ctx.enter_context(tc.tile_pool(name="v_ps", bufs=2, space="PSUM"))
        sc_ps = ctx.enter_context(tc.tile_pool(name="sc_ps", bufs=2, space="PSUM"))
        pv_ps = ctx.enter_context(tc.tile_pool(name="pv_ps", bufs=2, space="PSUM"))
        if mode != "bf16":
            nat_pool = ctx.enter_context(tc.tile_pool(name="nat", bufs=12))
            tp_ps = ctx.enter_context(tc.tile_pool(name="tp_ps", bufs=2, space="PSUM"))

        identity = const_pool.tile([P, P], f32, name="identity")
        make_identity(nc, identity[:])
        ones_col = const_pool.tile([P, 1], f32, name="ones_col")
        nc.gpsimd.memset(ones_col[:], 1.0)
        identity_r = const_pool.tile([P, P], f32r, name="identity_r")
        nc.vector.tensor_copy(identity_r[:], identity[:])
        id_mm = identity_r[:]

        # causal 0/1 masks for the 4 diagonal-straddling block offsets:
        # masks[m][r, c] = 1.0 iff c - r - 128*m >= 0
        masks = []
        for m in range(4):
            mk_raw = const_pool.tile([P, CH], f32, name=f"mask{m}_raw")
            nc.gpsimd.memset(mk_raw[:], 1.0)
            nc.gpsimd.affine_select(
                mk_raw[:], mk_raw[:],
                pattern=[[1, CH]],
                compare_op=mybir.AluOpType.is_ge,
                fill=0.0,
                base=-P * m,
                channel_multiplier=-1,
            )
            mk = const_pool.tile([P, CH], f32r, name=f"mask{m}")
            nc.vector.tensor_copy(mk[:], mk_raw[:])
            masks.append(mk)

        w_sb = {}
        for name, ap in (("wq", wq), ("wk", wk), ("wv", wv)):
            raw = const_pool.tile([P, KO, DO], f32, name=f"{name}_raw")
            nc.gpsimd.dma_start(raw[:], ap.rearrange("(ko ki) d -> ki ko d", ki=P))
            t = const_pool.tile([P, KO, DO], xdt, name=f"{name}_sb")
            nc.vector.tensor_copy(t[:], raw[:])
            w_sb[name] = t

        for xi in range(nx):
            # --- X^T tiles + projections for all three tensors ---
            pts = {}
            for tname, x_ap, wname in (("q", xq, "wq"), ("k", xk, "wk"), ("v", xv, "wv")):
                xt = xt_pool.tile([P, KO, S], xdt, name="xt", tag="xt")
                if mode == "bf16":
                    nc.sync.dma_start_transpose(xt[:], x_ap[xi])
                else:
                    for st in range(ST):
                        nat = nat_pool.tile([P, DK], f32, name="nat", tag="nat")
                        nc.sync.dma_start(nat[:], x_ap[xi, st * P:(st + 1) * P, :])
                        tp = tp_ps.tile([P, KO, P], f32, name="tp", tag="tp")
                        for ko in range(KO):
                            nc.tensor.transpose(
                                tp[:, ko, :], nat[:, ko * P:(ko + 1) * P], identity[:]
                            )
                        # one wide copy PSUM -> SBUF (rounds to f32r)
                        if st % 2 == 0:
                            nc.vector.tensor_copy(xt[:, :, st * P:(st + 1) * P], tp[:])
                        else:
                            nc.scalar.copy(xt[:, :, st * P:(st + 1) * P], tp[:])

                # projection: P^T = W^T X^T  -> [64, S] float32r
                pt = qkv_pool.tile([DO, S], f32r, name=f"{tname}T", tag=f"{tname}T")
                for c in range(NCH):
                    pp = proj_ps.tile([DO, CH], f32, name="pp", tag="pp")
                    for ko in range(KO):
                        nc.tensor.matmul(
                            pp[:],
                            w_sb[wname][:, ko, :],
                            xt[:, ko, c * CH:(c + 1) * CH],
                            start=(ko == 0),
                            stop=(ko == KO - 1),
                        )
                    if c % 2 == 0:
                        nc.vector.tensor_copy(pt[:, c * CH:(c + 1) * CH], pp[:])
                    else:
                        nc.scalar.copy(pt[:, c * CH:(c + 1) * CH], pp[:])
                pts[tname] = pt

            # --- V_ext tiles: [128 sk, 65] = [V tile | ones] ---
            vext = vext_pool.tile([P, ST, DO + 1], f32r, name="vext", tag="vext")
            nc.vector.tensor_copy(vext[:, :, DO], ones_col[:].to_broadcast([P, ST]))
            for i in range(ST):
                vp = v_ps.tile([P, DO], f32r, name="vp", tag="vp")
                nc.tensor.transpose(
                    vp[:], pts["v"][:, i * P:(i + 1) * P], id_mm[:DO, :DO]
                )
                nc.vector.tensor_copy(vext[:, i, :DO], vp[:])

            # --- attention ---
            outT = outT_pool.tile([DO + 1, S], f32, name="outT", tag="outT")
            for j in range(NCH):
                ilast = min(ST - 1, 4 * j + 3)
                pv = pv_ps.tile([DO + 1, CH], f32, name="pv", tag="pv")
                for i in range(ilast + 1):
                    sc = sc_ps.tile([P, CH], f32, name="sc", tag="sc")
                    nc.tensor.matmul(
                        sc[:],
                        pts["k"][:, i * P:(i + 1) * P],
                        pts["q"][:, j * CH:(j + 1) * CH],
                        start=True,
                        stop=True,
                    )
                    at = at_pool.tile([P, CH], f32r, name="at", tag="at")
                    nc.scalar.activation(
                        at[:], sc[:],
                        mybir.ActivationFunctionType.Exp,
                        scale=1.0 / 32.0,
                    )
                    if i >= 4 * j:  # block straddles the diagonal
                        nc.vector.tensor_tensor(
                            at[:], at[:], masks[i - 4 * j][:],
                            mybir.AluOpType.mult,
                        )
                    nc.tensor.matmul(
                        pv[:],
                        vext[:, i, :],
                        at[:],
                        start=(i == 0),
                        stop=(i == ilast),
                    )
                if j % 2 == 0:
                    nc.vector.tensor_copy(outT[:, j * CH:(j + 1) * CH], pv[:])
                else:
                    nc.scalar.copy(outT[:, j * CH:(j + 1) * CH], pv[:])
            nc.gpsimd.dma_start(out[xi], outT[:])

    nc.compile()
    return nc


_NC_CACHE: dict = {}


def _get_nc(nx: int = X, mode: str = MODE):
    key = (nx, mode)
    if key not in _NC_CACHE:
        _NC_CACHE[key] = build_nc(nx, mode)
    return _NC_CACHE[key]


def prep_x(x: np.ndarray, mode: str):
    """Host-side input prep for one [..., S, DK] tensor."""
    import ml_dtypes

    if mode == "bf16":
        return np.ascontiguousarray(x.astype(ml_dtypes.bfloat16))
    return np.ascontiguousarray(x.astype(np.float32))


def run_device(inputs: dict, mode: str = MODE, trace: bool = False, **kw):
    """Run the SPMD kernel on 8 cores; returns (per-core raw outs, BassKernelResults)."""
    from concourse import bass_utils

    xq = prep_x(np.asarray(inputs["inputs_for_queries"], np.float32), mode)
    xk = prep_x(np.asarray(inputs["inputs_for_keys"], np.float32), mode)
    xv = prep_x(np.asarray(inputs["inputs_for_values"], np.float32), mode)
    wq = np.ascontiguousarray(np.asarray(inputs["q_weight"], np.float32))
    wk = np.ascontiguousarray(np.asarray(inputs["k_weight"], np.float32))
    wv = np.ascontiguousarray(np.asarray(inputs["v_weight"], np.float32))

    nc = _get_nc(X, mode)
    in_maps = [
        {"xq": xq[b], "xk": xk[b], "xv": xv[b], "wq": wq, "wk": wk, "wv": wv}
        for b in range(NCORES)
    ]
    res = bass_utils.run_bass_kernel_spmd(
        nc, in_maps, core_ids=list(range(NCORES)), trace=trace, **kw
    )
    outs = [res.results[b]["out"] for b in range(NCORES)]
    return outs, res


def finalize(outs) -> np.ndarray:
    """Host-side: normalize by the softmax denominator and un-transpose."""
    full = np.stack(outs, axis=0)                      # [B, X, 65, S]
    o = full[:, :, :DO, :] / full[:, :, DO:DO + 1, :]  # [B, X, 64, S]
    return np.ascontiguousarray(o.transpose(0, 1, 3, 2)).astype(np.float32)


def kernel(**inputs) -> np.ndarray:
    outs, _ = run_device(inputs)
    return finalize(outs)


if __name__ == "__main__":
    nc = build_nc(1)
    print("built ok")


# revision 16
# speedup vs baseline: 1.2575x; 1.0009x over previous
"""Trainium2 Bass kernel for nn_AttentionHeadRankFour.

Computes, for inputs Xq, Xk, Xv [B=8, X=8, S=1024, D=512] and weights
Wq, Wk, Wv [512, 64]:
    Q = Xq Wq; K = Xk Wk; V = Xv Wv
    scores = Q K^T / sqrt(S), causal mask, softmax
    out = attn V                                  -> [8, 8, 1024, 64]

Sharding: data-parallel over batch B across 8 NeuronCores (core c gets b=c).
Weights replicated.

Per-core device program, in "transposed land" so the attention matrix never
needs an on-chip transpose:
  - X^T [k, s] tiles on chip (per-mode, see below)
  - projections with W as the stationary operand -> Q^T, K^T, V^T [64, 1024]
  - scores^T[sk,sq] = K @ Q^T block matmuls (float32r), fully-masked blocks
    skipped
  - attn^T = exp(scores^T/32) via ACT (the 1/sqrt(S) folds into the
    activation scale), causal edge blocks fixed by a gpsimd affine_select
  - PV: lhsT = V_ext [sk-tile, 65] (V tile plus a ones column) so the
    softmax denominator accumulates as row 64 of the output psum
  - out^T [65, 1024] DMA'd out; host divides rows 0:64 by row 64 and
    transposes to [1024, 64]

Modes (how X^T is produced / projection precision):
  - "f32r": inputs fp32, natural loads + PE transposes (exact), projections
    in float32r (fp32 storage, ~2^-14 multiply rounding, fp32 accumulate).
  - "bf16": inputs pre-cast to bf16 on host in [KO, S, 128] k-chunked
    layout; X^T produced directly by XBAR DMA-transpose loads (no PE
    transposes, no PSUM->SBUF copies); projections in bf16; attention
    stays float32r.
"""

import numpy as np

P = 128
S = 1024
DK = 512
DO = 64
KO = DK // P          # 4 k-tiles of 128
ST = S // P           # 8 s-tiles of 128
CH = 512              # sq chunk width (psum bank)
NCH = S // CH         # 2
NCORES = 8
B = 8
X = 8

MODE = "bf16"          # "f32r" or "bf16"


def build_nc(nx: int = X, mode: str = MODE):
    """Build the single-core Bass program processing nx x-slices."""
    from contextlib import ExitStack

    import concourse.tile as tile
    from concourse import bacc, mybir
    from concourse.masks import make_identity

    f32 = mybir.dt.float32
    f32r = mybir.dt.float32r
    bf16 = mybir.dt.bfloat16

    nc = bacc.Bacc("TRN2", target_bir_lowering=False, debug=False)

    if mode == "bf16":
        xq = nc.dram_tensor("xq", [nx, S, DK], bf16, kind="ExternalInput").ap()
        xk = nc.dram_tensor("xk", [nx, S, DK], bf16, kind="ExternalInput").ap()
        xv = nc.dram_tensor("xv", [nx, S, DK], bf16, kind="ExternalInput").ap()
        xdt = bf16
    else:
        xq = nc.dram_tensor("xq", [nx, S, DK], f32, kind="ExternalInput").ap()
        xk = nc.dram_tensor("xk", [nx, S, DK], f32, kind="ExternalInput").ap()
        xv = nc.dram_tensor("xv", [nx, S, DK], f32, kind="ExternalInput").ap()
        xdt = f32r
    wq = nc.dram_tensor("wq", [DK, DO], f32, kind="ExternalInput").ap()
    wk = nc.dram_tensor("wk", [DK, DO], f32, kind="ExternalInput").ap()
    wv = nc.dram_tensor("wv", [DK, DO], f32, kind="ExternalInput").ap()
    out = nc.dram_tensor("out", [nx, DO + 1, S], f32, kind="ExternalOutput").ap()

    with tile.TileContext(nc) as tc, ExitStack() as ctx:
        const_pool = ctx.enter_context(tc.tile_pool(name="const", bufs=1))
        xt_pool = ctx.enter_context(tc.tile_pool(name="xt", bufs=6))
        qkv_pool = ctx.enter_context(tc.tile_pool(name="qkv", bufs=2))
        vext_pool = ctx.enter_context(tc.tile_pool(name="vext", bufs=2))
        at_pool = ctx.enter_context(tc.tile_pool(name="at", bufs=6))
        outT_pool = ctx.enter_context(tc.tile_pool(name="outT", bufs=2))

        proj_ps = ctx.enter_context(tc.tile_pool(name="proj_ps", bufs=2, space="PSUM"))
        v_ps = ctx.enter_context(tc.tile_pool(name="v_ps", bufs=1, space="PSUM"))
        sc_ps = ctx.enter_context(tc.tile_pool(name="sc_ps", bufs=3, space="PSUM"))
        pv_ps = ctx.enter_context(tc.tile_pool(name="pv_ps", bufs=2, space="PSUM"))
        if mode != "bf16":
            nat_pool = ctx.enter_context(tc.tile_pool(name="nat", bufs=12))
            tp_ps = ctx.enter_context(tc.tile_pool(name="tp_ps", bufs=2, space="PSUM"))

        identity = const_pool.tile([P, P], f32, name="identity")
        make_identity(nc, identity[:])
        ones_col = const_pool.tile([P, 1], f32, name="ones_col")
        nc.gpsimd.memset(ones_col[:], 1.0)
        identity_r = const_pool.tile([P, P], f32r, name="identity_r")
        nc.vector.tensor_copy(identity_r[:], identity[:])
        id_mm = identity_r[:]

        # causal 0/1 masks for the 4 diagonal-straddling block offsets:
        # masks[m][r, c] = 1.0 iff c - r - 128*m >= 0
        masks = []
        for m in range(4):
            mk_raw = const_pool.tile([P, CH], f32, name=f"mask{m}_raw")
            nc.gpsimd.memset(mk_raw[:], 1.0)
            nc.gpsimd.affine_select(
                mk_raw[:], mk_raw[:],
                pattern=[[1, CH]],
                compare_op=mybir.AluOpType.is_ge,
                fill=0.0,
                base=-P * m,
                channel_multiplier=-1,
            )
            mk = const_pool.tile([P, CH], f32r, name=f"mask{m}")
            nc.vector.tensor_copy(mk[:], mk_raw[:])
            masks.append(mk)

        w_sb = {}
        for name, ap in (("wq", wq), ("wk", wk), ("wv", wv)):
            raw = const_pool.tile([P, KO, DO], f32, name=f"{name}_raw")
            nc.gpsimd.dma_start(raw[:], ap.rearrange("(ko ki) d -> ki ko d", ki=P))
            t = const_pool.tile([P, KO, DO], xdt, name=f"{name}_sb")
            nc.vector.tensor_copy(t[:], raw[:])
            w_sb[name] = t

        for xi in range(nx):
            # --- X^T tiles + projections for all three tensors ---
            pts = {}
            for tname, x_ap, wname in (("q", xq, "wq"), ("k", xk, "wk"), ("v", xv, "wv")):
                xt = xt_pool.tile([P, KO, S], xdt, name="xt", tag="xt")
                if mode == "bf16":
                    nc.sync.dma_start_transpose(xt[:], x_ap[xi])
                else:
                    for st in range(ST):
                        nat = nat_pool.tile([P, DK], f32, name="nat", tag="nat")
                        nc.sync.dma_start(nat[:], x_ap[xi, st * P:(st + 1) * P, :])
                        tp = tp_ps.tile([P, KO, P], f32, name="tp", tag="tp")
                        for ko in range(KO):
                            nc.tensor.transpose(
                                tp[:, ko, :], nat[:, ko * P:(ko + 1) * P], identity[:]
                            )
                        # one wide copy PSUM -> SBUF (rounds to f32r)
                        if st % 2 == 0:
                            nc.vector.tensor_copy(xt[:, :, st * P:(st + 1) * P], tp[:])
                        else:
                            nc.scalar.copy(xt[:, :, st * P:(st + 1) * P], tp[:])

                # projection: P^T = W^T X^T -> [64, S] float32r.
                # Col-packed: ko pairs run concurrently in PE col groups
                # (out partitions 0:64 and 64:128 of one psum bank); the two
                # partial sums are added during the PSUM->SBUF move. Rows
                # 64:128 of pt duplicate 0:64 for the row-packed scores
                # matmuls.
                pt = qkv_pool.tile([DO, S], f32r, name=f"{tname}T", tag=f"{tname}T")
                for c in range(NCH):
                    pp = proj_ps.tile([DO, CH], f32, name="pp", tag="pp")
                    for ko in range(KO):
                        nc.tensor.matmul(
                            pp[:],
                            w_sb[wname][:, ko, :],
                            xt[:, ko, c * CH:(c + 1) * CH],
                            start=(ko == 0),
                            stop=(ko == KO - 1),
                        )
                    if c % 2 == 0:
                        nc.vector.tensor_copy(pt[:, c * CH:(c + 1) * CH], pp[:])
                    else:
                        nc.scalar.copy(pt[:, c * CH:(c + 1) * CH], pp[:])
                pts[tname] = pt

            # --- V_ext tiles: [128 sk, 65] = [V tile | ones] ---
            vext = vext_pool.tile([P, ST, DO + 1], f32r, name="vext", tag="vext")
            nc.vector.tensor_copy(vext[:, :, DO], ones_col[:].to_broadcast([P, ST]))
            for i in range(ST):
                vp = v_ps.tile([P, DO], f32r, name="vp", tag="vp")
                nc.tensor.transpose(
                    vp[:], pts["v"][:, i * P:(i + 1) * P], id_mm[:DO, :DO]
                )
                nc.vector.tensor_copy(vext[:, i, :DO], vp[:])

            # --- attention ---
            outT = outT_pool.tile([DO + 1, S], f32, name="outT", tag="outT")
            for j in range(NCH):
                ilast = min(ST - 1, 4 * j + 3)
                pv = pv_ps.tile([DO + 1, CH], f32, name="pv", tag="pv")
                for i in range(ilast + 1):
                    sc = sc_ps.tile([P, CH], f32, name="sc", tag="sc")
                    nc.tensor.matmul(
                        sc[:],
                        pts["k"][:, i * P:(i + 1) * P],
                        pts["q"][:, j * CH:(j + 1) * CH],
                        start=True,
                        stop=True,
                    )
                    at = at_pool.tile([P, CH], f32r, name="at", tag="at")
                    nc.scalar.activation(
                        at[:], sc[:],
                        mybir.ActivationFunctionType.Exp,
                        scale=1.0 / 32.0,
                    )
                    if i >= 4 * j:  # block straddles the diagonal
                        nc.vector.tensor_tensor(
                            at[:], at[:], masks[i - 4 * j][:],
                            mybir.AluOpType.mult,
                        )
                    nc.tensor.matmul(
                        pv[:],
                        vext[:, i, :],
                        at[:],
                        start=(i == 0),
                        stop=(i == ilast),
                    )
                if j % 2 == 0:
                    nc.vector.tensor_copy(outT[:, j * CH:(j + 1) * CH], pv[:])
                else:
                    nc.scalar.copy(outT[:, j * CH:(j + 1) * CH], pv[:])
            nc.gpsimd.dma_start(out[xi], outT[:])

    nc.compile()
    return nc


_NC_CACHE: dict = {}


def _get_nc(nx: int = X, mode: str = MODE):
    key = (nx, mode)
    if key not in _NC_CACHE:
        _NC_CACHE[key] = build_nc(nx, mode)
    return _NC_CACHE[key]


def prep_x(x: np.ndarray, mode: str):
    """Host-side input prep for one [..., S, DK] tensor."""
    import ml_dtypes

    if mode == "bf16":
        return np.ascontiguousarray(x.astype(ml_dtypes.bfloat16))
    return np.ascontiguousarray(x.astype(np.float32))


def run_device(inputs: dict, mode: str = MODE, trace: bool = False, **kw):
    """Run the SPMD kernel on 8 cores; returns (per-core raw outs, BassKernelResults)."""
    from concourse import bass_utils

    xq = prep_x(np.asarray(inputs["inputs_for_queries"], np.float32), mode)
    xk = prep_x(np.asarray(inputs["inputs_for_keys"], np.float32), mode)
    xv = prep_x(np.asarray(inputs["inputs_for_values"], np.float32), mode)
    wq = np.ascontiguousarray(np.asarray(inputs["q_weight"], np.float32))
    wk = np.ascontiguousarray(np.asarray(inputs["k_weight"], np.float32))
    wv = np.ascontiguousarray(np.asarray(inputs["v_weight"], np.float32))

    nc = _get_nc(X, mode)
    in_maps = [
        {"xq": xq[b], "xk": xk[b], "xv": xv[b], "wq": wq, "wk": wk, "wv": wv}
        for b in range(NCORES)
    ]
    res = bass_utils.run_bass_kernel_spmd(
        nc, in_maps, core_ids=list(range(NCORES)), trace=trace, **kw
    )
    outs = [res.results[b]["out"] for b in range(NCORES)]
    return outs, res


def finalize(outs) -> np.ndarray:
    """Host-side: normalize by the softmax denominator and un-transpose."""
    full = np.stack(outs, axis=0)                      # [B, X, 65, S]
    o = full[:, :, :DO, :] / full[:, :, DO:DO + 1, :]  # [B, X, 64, S]
    return np.ascontiguousarray(o.transpose(0, 1, 3, 2)).astype(np.float32)


def kernel(**inputs) -> np.ndarray:
    outs, _ = run_device(inputs)
    return finalize(outs)


if __name__ == "__main__":
    nc = build_nc(1)
    print("built ok")


# revision 17
# speedup vs baseline: 1.3586x; 1.0804x over previous
"""Trainium2 Bass kernel for nn_AttentionHeadRankFour.

Computes, for inputs Xq, Xk, Xv [B=8, X=8, S=1024, D=512] and weights
Wq, Wk, Wv [512, 64]:
    Q = Xq Wq; K = Xk Wk; V = Xv Wv
    scores = Q K^T / sqrt(S), causal mask, softmax
    out = attn V                                  -> [8, 8, 1024, 64]

Sharding: data-parallel over batch B across 8 NeuronCores (core c gets b=c).
Weights replicated.

Per-core device program, in "transposed land" so the attention matrix never
needs an on-chip transpose:
  - X^T [k, s] tiles on chip (per-mode, see below)
  - projections with W as the stationary operand -> Q^T, K^T, V^T [64, 1024]
  - scores^T[sk,sq] = K @ Q^T block matmuls (float32r), fully-masked blocks
    skipped
  - attn^T = exp(scores^T/32) via ACT (the 1/sqrt(S) folds into the
    activation scale), causal edge blocks fixed by a gpsimd affine_select
  - PV: lhsT = V_ext [sk-tile, 65] (V tile plus a ones column) so the
    softmax denominator accumulates as row 64 of the output psum
  - out^T [65, 1024] DMA'd out; host divides rows 0:64 by row 64 and
    transposes to [1024, 64]

Modes (how X^T is produced / projection precision):
  - "f32r": inputs fp32, natural loads + PE transposes (exact), projections
    in float32r (fp32 storage, ~2^-14 multiply rounding, fp32 accumulate).
  - "bf16": inputs pre-cast to bf16 on host in [KO, S, 128] k-chunked
    layout; X^T produced directly by XBAR DMA-transpose loads (no PE
    transposes, no PSUM->SBUF copies); projections in bf16; attention
    stays float32r.
"""

import numpy as np

P = 128
S = 1024
DK = 512
DO = 64
KO = DK // P          # 4 k-tiles of 128
ST = S // P           # 8 s-tiles of 128
CH = 512              # sq chunk width (psum bank)
NCH = S // CH         # 2
NCORES = 8
B = 8
X = 8

MODE = "bf16"          # "f32r" or "bf16"


def build_nc(nx: int = X, mode: str = MODE):
    """Build the single-core Bass program processing nx x-slices."""
    from contextlib import ExitStack

    import concourse.tile as tile
    from concourse import bacc, mybir
    from concourse.masks import make_identity

    f32 = mybir.dt.float32
    f32r = mybir.dt.float32r
    bf16 = mybir.dt.bfloat16

    nc = bacc.Bacc("TRN2", target_bir_lowering=False, debug=False)

    if mode == "bf16":
        xq = nc.dram_tensor("xq", [nx, S, DK], bf16, kind="ExternalInput").ap()
        xk = nc.dram_tensor("xk", [nx, S, DK], bf16, kind="ExternalInput").ap()
        xv = nc.dram_tensor("xv", [nx, S, DK], bf16, kind="ExternalInput").ap()
        xdt = bf16
    else:
        xq = nc.dram_tensor("xq", [nx, S, DK], f32, kind="ExternalInput").ap()
        xk = nc.dram_tensor("xk", [nx, S, DK], f32, kind="ExternalInput").ap()
        xv = nc.dram_tensor("xv", [nx, S, DK], f32, kind="ExternalInput").ap()
        xdt = f32r
    wq = nc.dram_tensor("wq", [DK, DO], f32, kind="ExternalInput").ap()
    wk = nc.dram_tensor("wk", [DK, DO], f32, kind="ExternalInput").ap()
    wv = nc.dram_tensor("wv", [DK, DO], f32, kind="ExternalInput").ap()
    out = nc.dram_tensor("out", [nx, DO + 1, S], f32, kind="ExternalOutput").ap()

    with tile.TileContext(nc) as tc, ExitStack() as ctx:
        const_pool = ctx.enter_context(tc.tile_pool(name="const", bufs=1))
        xt_pool = ctx.enter_context(tc.tile_pool(name="xt", bufs=6))
        qkv_pool = ctx.enter_context(tc.tile_pool(name="qkv", bufs=2))
        vext_pool = ctx.enter_context(tc.tile_pool(name="vext", bufs=2))
        at_pool = ctx.enter_context(tc.tile_pool(name="at", bufs=6))
        outT_pool = ctx.enter_context(tc.tile_pool(name="outT", bufs=2))

        proj_ps = ctx.enter_context(tc.tile_pool(name="proj_ps", bufs=2, space="PSUM"))
        v_ps = ctx.enter_context(tc.tile_pool(name="v_ps", bufs=1, space="PSUM"))
        sc_ps = ctx.enter_context(tc.tile_pool(name="sc_ps", bufs=3, space="PSUM"))
        pv_ps = ctx.enter_context(tc.tile_pool(name="pv_ps", bufs=2, space="PSUM"))
        if mode != "bf16":
            nat_pool = ctx.enter_context(tc.tile_pool(name="nat", bufs=12))
            tp_ps = ctx.enter_context(tc.tile_pool(name="tp_ps", bufs=2, space="PSUM"))

        identity = const_pool.tile([P, P], f32, name="identity")
        make_identity(nc, identity[:])
        ones_col = const_pool.tile([P, 1], f32, name="ones_col")
        nc.gpsimd.memset(ones_col[:], 1.0)
        identity_r = const_pool.tile([P, P], f32r, name="identity_r")
        nc.vector.tensor_copy(identity_r[:], identity[:])
        id_mm = identity_r[:]

        # causal 0/1 masks for the 4 diagonal-straddling block offsets:
        # masks[m][r, c] = 1.0 iff c - r - 128*m >= 0
        masks = []
        for m in range(4):
            mk_raw = const_pool.tile([P, CH], f32, name=f"mask{m}_raw")
            nc.gpsimd.memset(mk_raw[:], 1.0)
            nc.gpsimd.affine_select(
                mk_raw[:], mk_raw[:],
                pattern=[[1, CH]],
                compare_op=mybir.AluOpType.is_ge,
                fill=0.0,
                base=-P * m,
                channel_multiplier=-1,
            )
            mk = const_pool.tile([P, CH], f32r, name=f"mask{m}")
            nc.vector.tensor_copy(mk[:], mk_raw[:])
            masks.append(mk)

        w_sb = {}
        for name, ap in (("wq", wq), ("wk", wk), ("wv", wv)):
            raw = const_pool.tile([P, KO, DO], f32, name=f"{name}_raw")
            nc.gpsimd.dma_start(raw[:], ap.rearrange("(ko ki) d -> ki ko d", ki=P))
            t = const_pool.tile([P, KO, DO], xdt, name=f"{name}_sb")
            nc.vector.tensor_copy(t[:], raw[:])
            w_sb[name] = t

        for xi in range(nx):
            # --- X^T tiles + projections for all three tensors ---
            pts = {}
            for tname, x_ap, wname in (("q", xq, "wq"), ("k", xk, "wk"), ("v", xv, "wv")):
                xt = xt_pool.tile([P, KO, S], xdt, name="xt", tag="xt")
                if mode == "bf16":
                    nc.sync.dma_start_transpose(xt[:], x_ap[xi])
                else:
                    for st in range(ST):
                        nat = nat_pool.tile([P, DK], f32, name="nat", tag="nat")
                        nc.sync.dma_start(nat[:], x_ap[xi, st * P:(st + 1) * P, :])
                        tp = tp_ps.tile([P, KO, P], f32, name="tp", tag="tp")
                        for ko in range(KO):
                            nc.tensor.transpose(
                                tp[:, ko, :], nat[:, ko * P:(ko + 1) * P], identity[:]
                            )
                        # one wide copy PSUM -> SBUF (rounds to f32r)
                        if st % 2 == 0:
                            nc.vector.tensor_copy(xt[:, :, st * P:(st + 1) * P], tp[:])
                        else:
                            nc.scalar.copy(xt[:, :, st * P:(st + 1) * P], tp[:])

                # projection: P^T = W^T X^T -> [64, S] float32r.
                # Col-packed: ko pairs run concurrently in PE col groups
                # (out partitions 0:64 and 64:128 of one psum bank); the two
                # partial sums are added during the PSUM->SBUF move. Rows
                # 64:128 of pt duplicate 0:64 for the row-packed scores
                # matmuls.
                pt = qkv_pool.tile([DO, S], f32r, name=f"{tname}T", tag=f"{tname}T")
                for c in range(NCH):
                    pp = proj_ps.tile([DO, CH], f32, name="pp", tag="pp")
                    for ko in range(KO):
                        nc.tensor.matmul(
                            pp[:],
                            w_sb[wname][:, ko, :],
                            xt[:, ko, c * CH:(c + 1) * CH],
                            start=(ko == 0),
                            stop=(ko == KO - 1),
                        )
                    if c % 2 == 0:
                        nc.vector.tensor_copy(pt[:, c * CH:(c + 1) * CH], pp[:])
                    else:
                        nc.scalar.copy(pt[:, c * CH:(c + 1) * CH], pp[:])
                pts[tname] = pt

            # --- V_ext tiles: [128 sk, 65] = [V tile | ones] ---
            vext = vext_pool.tile([P, ST, DO + 1], f32r, name="vext", tag="vext")
            nc.vector.tensor_copy(vext[:, :, DO], ones_col[:].to_broadcast([P, ST]))
            for i in range(ST):
                vp = v_ps.tile([P, DO], f32r, name="vp", tag="vp")
                nc.tensor.transpose(
                    vp[:], pts["v"][:, i * P:(i + 1) * P], id_mm[:DO, :DO]
                )
                nc.vector.tensor_copy(vext[:, i, :DO], vp[:])

            # --- attention (sc/exp/mask pipelined one step ahead of pv) ---
            outT = outT_pool.tile([DO + 1, S], f32, name="outT", tag="outT")
            for j in range(NCH):
                ilast = min(ST - 1, 4 * j + 3)
                pv = pv_ps.tile([DO + 1, CH], f32, name="pv", tag="pv")

                def make_at(i, j=j):
                    sc = sc_ps.tile([P, CH], f32, name="sc", tag="sc")
                    nc.tensor.matmul(
                        sc[:],
                        pts["k"][:, i * P:(i + 1) * P],
                        pts["q"][:, j * CH:(j + 1) * CH],
                        start=True,
                        stop=True,
                    )
                    at = at_pool.tile([P, CH], f32r, name="at", tag="at")
                    nc.scalar.activation(
                        at[:], sc[:],
                        mybir.ActivationFunctionType.Exp,
                        scale=1.0 / 32.0,
                    )
                    if i >= 4 * j:  # block straddles the diagonal
                        m = i - 4 * j
                        w = P * (m + 1)  # columns right of this are all-keep
                        nc.vector.tensor_tensor(
                            at[:, :w], at[:, :w], masks[m][:, :w],
                            mybir.AluOpType.mult,
                        )
                    return at

                prev = make_at(0)
                for i in range(1, ilast + 1):
                    cur = make_at(i)
                    nc.tensor.matmul(
                        pv[:], vext[:, i - 1, :], prev[:],
                        start=(i - 1 == 0), stop=False,
                    )
                    prev = cur
                nc.tensor.matmul(
                    pv[:], vext[:, ilast, :], prev[:],
                    start=(ilast == 0), stop=True,
                )
                if j % 2 == 0:
                    nc.vector.tensor_copy(outT[:, j * CH:(j + 1) * CH], pv[:])
                else:
                    nc.scalar.copy(outT[:, j * CH:(j + 1) * CH], pv[:])
            nc.gpsimd.dma_start(out[xi], outT[:])

    nc.compile()
    return nc


_NC_CACHE: dict = {}


def _get_nc(nx: int = X, mode: str = MODE):
    key = (nx, mode)
    if key not in _NC_CACHE:
        _NC_CACHE[key] = build_nc(nx, mode)
    return _NC_CACHE[key]


def prep_x(x: np.ndarray, mode: str):
    """Host-side input prep for one [..., S, DK] tensor."""
    import ml_dtypes

    if mode == "bf16":
        return np.ascontiguousarray(x.astype(ml_dtypes.bfloat16))
    return np.ascontiguousarray(x.astype(np.float32))


def run_device(inputs: dict, mode: str = MODE, trace: bool = False, **kw):
    """Run the SPMD kernel on 8 cores; returns (per-core raw outs, BassKernelResults)."""
    from concourse import bass_utils

    xq = prep_x(np.asarray(inputs["inputs_for_queries"], np.float32), mode)
    xk = prep_x(np.asarray(inputs["inputs_for_keys"], np.float32), mode)
    xv = prep_x(np.asarray(inputs["inputs_for_values"], np.float32), mode)
    wq = np.ascontiguousarray(np.asarray(inputs["q_weight"], np.float32))
    wk = np.ascontiguousarray(np.asarray(inputs["k_weight"], np.float32))
    wv = np.ascontiguousarray(np.asarray(inputs["v_weight"], np.float32))

    nc = _get_nc(X, mode)
    in_maps = [
        {"xq": xq[b], "xk": xk[b], "xv": xv[b], "wq": wq, "wk": wk, "wv": wv}
        for b in range(NCORES)
    ]
    res = bass_utils.run_bass_kernel_spmd(
        nc, in_maps, core_ids=list(range(NCORES)), trace=trace, **kw
    )
    outs = [res.results[b]["out"] for b in range(NCORES)]
    return outs, res


def finalize(outs) -> np.ndarray:
    """Host-side: normalize by the softmax denominator and un-transpose."""
    full = np.stack(outs, axis=0)                      # [B, X, 65, S]
    o = full[:, :, :DO, :] / full[:, :, DO:DO + 1, :]  # [B, X, 64, S]
    return np.ascontiguousarray(o.transpose(0, 1, 3, 2)).astype(np.float32)


def kernel(**inputs) -> np.ndarray:
    outs, _ = run_device(inputs)
    return finalize(outs)


if __name__ == "__main__":
    nc = build_nc(1)
    print("built ok")
